# revision 52
# baseline (speedup 1.0000x reference)
"""Causal self-attention (GQA + RoPE) Trainium2 Bass kernel, 8 NeuronCores.

Problem: B=2, T=2048, C=2048, n_head=16, n_kv_head=4, head_dim=128.

Sharding: 2-way batch DP x 4-way head TP. Core c = 4*b + g handles batch b,
kv head g, q heads [4g, 4g+4). wq/wk/wv column-sharded per head group, wo
row-sharded; per-core partial outputs are summed on the host (the gather /
unshard step), so no on-device collective is needed.

v3: fully fused pipeline. Per 512-col t-chunk i the PE emission order is
  proj(i) -> V-transpose(i) -> outproj(i-1) -> attention(i)
so the tensor engine never crosses a phase barrier (keeps HAM warm).
RoPE is software-pipelined (swap-DMA issued at evac, cos/sin muls one pass
later) so the DVE never blocks on DMA latency and PSUM banks recycle
promptly; attn(3) holds back 3 outproj(2) fillers to cover the final
flush->reciprocal->mul chain; the last chunk's stores are per-co DMAs.
Projection runs one output at a time (k, q0..q3, v: 16-chunk accumulation
chains in a single PSUM bank each) so projections need only the 2 shared
"generic" PSUM banks; attention uses 2-bank score pairs (one exp per block
pair), accumulating den (ones-matmul) and O over s-blocks; softmax denom
reciprocal via the fast approx DVE op. All DRAM inputs are host-pre-tiled
to match SBUF layouts so every DMA is contiguous; outputs are fp16
partials summed on the host.
"""

import sys

sys.path.insert(0, "/opt/trn_rl_repo")

import numpy as np

import concourse.bass as bass
import concourse.mybir as mybir
import concourse.tile as tile
from concourse import bacc
from concourse.bass_utils import run_bass_kernel_spmd
from concourse.masks import make_identity

F32 = mybir.dt.float32
F16 = mybir.dt.float16
FP8 = mybir.dt.float8e4
AF = mybir.ActivationFunctionType
DR = mybir.MatmulPerfMode.DoubleRow
DP = mybir.MatmulPerfMode.DoublePixel

B, T, C = 2, 2048, 2048
N_HEAD, N_KV_HEAD = 16, 4
HD = 128                 # head dim
QH = 4                   # q heads per core
TQ = 512                 # t-chunk
NT = T // TQ             # 4 t-chunks
CK = C // 128            # 16 contraction chunks of 128
SCALE = 1.0 / float(np.sqrt(HD))
# exp is computed as exp(score*SCALE - ln8) = exp(score*SCALE)/8; the /8
# cancels in O/den but keeps the fp8e4 den copy far from TRN-fp8's +-240
# max (values >=248 cast to inf -> NaN through the reciprocal).
EXP_BIAS = -float(np.log(8.0))
MASK_NEG = -1e30

_CACHE = {}


def _build_nc():
    nc = bacc.Bacc("TRN2", target_bir_lowering=False, debug=False, num_devices=8)

    # All inputs pre-tiled on host so DRAM layout == SBUF layout.
    xH = nc.dram_tensor("xH", [NT, 128, CK, TQ], F16, kind="ExternalInput").ap()
    wqH = nc.dram_tensor("wqH", [128, CK, QH * HD], F16, kind="ExternalInput").ap()
    wkH = nc.dram_tensor("wkH", [128, CK, HD], F16, kind="ExternalInput").ap()
    wvH = nc.dram_tensor("wvH", [128, CK, HD], F16, kind="ExternalInput").ap()
    woH = nc.dram_tensor("woH", [128, CK, QH * HD], F16, kind="ExternalInput").ap()
    cosH = nc.dram_tensor("cosH", [HD, T], F16, kind="ExternalInput").ap()
    sinH = nc.dram_tensor("sinH", [HD, T], F16, kind="ExternalInput").ap()
    # fp8 copies for the chunk>=1 q/k projections (x pre-scaled by 4, w by
    # 256 so weights sit in fp8e4's normal range; evac divides by 1024)
    x8H = nc.dram_tensor("x8H", [NT, 128, CK, TQ], FP8, kind="ExternalInput").ap()
    wq8H = nc.dram_tensor("wq8H", [128, CK, QH * HD], FP8,
                          kind="ExternalInput").ap()
    wk8H = nc.dram_tensor("wk8H", [128, CK, HD], FP8, kind="ExternalInput").ap()
    outX = nc.dram_tensor("outX", [NT, 128, CK, TQ], F16, kind="ExternalOutput").ap()

    with tile.TileContext(nc) as tc:
        _emit(nc, tc, xH, wqH, wkH, wvH, woH, cosH, sinH,
              x8H, wq8H, wk8H, outX)

    nc.compile()
    return nc


PROJ_UNSCALE = 1.0 / 1024.0


def _emit(nc, tc, xH, wqH, wkH, wvH, woH, cosH, sinH, x8H, wq8H, wk8H, outX):
    import contextlib

    ctx = contextlib.ExitStack()
    with ctx:
        singles = ctx.enter_context(tc.tile_pool(name="singles", bufs=1))

        # ---- resident tiles ----
        wq_sb = singles.tile([128, CK, QH * HD], F16)
        wk_sb = singles.tile([128, CK, HD], F16)
        wv_sb = singles.tile([128, CK, HD], F16)
        wo_sb = singles.tile([128, CK, QH * HD], F16)
        wq8_sb = singles.tile([128, CK, QH * HD], FP8)
        wk8_sb = singles.tile([128, CK, HD], FP8)
        cos_sb = singles.tile([HD, T], F16)
        sin_sb = singles.tile([HD, T], F16)

        qT_sb = singles.tile([128, QH, T], F16)    # per head [dq, t], RoPE'd
        kT_sb = singles.tile([128, T], F16)        # [dk, t], RoPE'd
        v_sb = singles.tile([128, CK, HD], F16)    # [s in blk, (blk, dv)]
        oT_sb = singles.tile([128, QH, T], F16)    # per head [dv, t] normalized

        ident = singles.tile([128, 128], F32)
        cmask = singles.tile([128, 128], F32)
        ones_sq = singles.tile([128, 128], F16)
        ones2_8 = singles.tile([128, 2, 128], FP8)
        ebias = singles.tile([128, 1], F32)

        # ---- pools ----
        xpool = ctx.enter_context(tc.tile_pool(name="xpool", bufs=2))
        x8pool = ctx.enter_context(tc.tile_pool(name="x8pool", bufs=2))
        ppool = ctx.enter_context(tc.tile_pool(name="ppool", bufs=3))
        vtsb = ctx.enter_context(tc.tile_pool(name="vtsb", bufs=2))
        rope = ctx.enter_context(tc.tile_pool(name="rope", bufs=3))
        invp = ctx.enter_context(tc.tile_pool(name="invp", bufs=2))
        outsb = ctx.enter_context(tc.tile_pool(name="outsb", bufs=2))
        ps_s = ctx.enter_context(tc.tile_pool(name="ps_s", bufs=2, space="PSUM"))
        ps_d = ctx.enter_context(tc.tile_pool(name="ps_d", bufs=1, space="PSUM"))
        ps_o = ctx.enter_context(tc.tile_pool(name="ps_o", bufs=1, space="PSUM"))
        ps_g = ctx.enter_context(tc.tile_pool(name="ps_g", bufs=2, space="PSUM"))

        # ---- startup DMAs, criticality-ordered ----
        x_t = [None] * NT
        x8_t = [None] * NT

        def load_x(i):
            x_t[i] = xpool.tile([128, CK, TQ], F16, tag="x", name=f"x{i}")
            for a in range(2):
                nc.sync.dma_start(out=x_t[i][:, 8 * a:8 * a + 8, :],
                                  in_=xH[i, :, 8 * a:8 * a + 8, :])

        def load_x8(i):
            x8_t[i] = x8pool.tile([128, CK, TQ], FP8, tag="x8", name=f"x8_{i}")
            for a in range(2):
                nc.sync.dma_start(out=x8_t[i][:, 8 * a:8 * a + 8, :],
                                  in_=x8H[i, :, 8 * a:8 * a + 8, :])

        # Startup DMAs: each dma_start costs ~600ns of ISSUE time on its
        # engine (transfers then run async), and nothing issues before the
        # engine programs load (~7us).  So: few, coarse pieces, split across
        # BOTH hw-DGE engines (sync + scalar) so the two issue queues run in
        # parallel.  sync gets the critical-path x0/wk; scalar gets wq and
        # the rest.
        nc.sync.dma_start(out=wk_sb, in_=wkH)
        x_t[0] = xpool.tile([128, CK, TQ], F16, tag="x", name="x0")
        for a in range(4):
            nc.sync.dma_start(out=x_t[0][:, 4 * a:4 * a + 4, :],
                              in_=xH[0, :, 4 * a:4 * a + 4, :])
        for a in range(2):
            nc.scalar.dma_start(out=wq_sb[:, 8 * a:8 * a + 8, :],
                                in_=wqH[:, 8 * a:8 * a + 8, :])
        nc.scalar.dma_start(out=cos_sb, in_=cosH)
        nc.scalar.dma_start(out=sin_sb, in_=sinH)
        nc.scalar.dma_start(out=wv_sb, in_=wvH)
        nc.scalar.dma_start(out=wq8_sb, in_=wq8H)
        nc.scalar.dma_start(out=wk8_sb, in_=wk8H)
        load_x8(1)  # needed by attn(0)'s chunk-1 k/q0 fillers
        load_x(1)
        nc.sync.dma_start(out=wo_sb, in_=woH)

        make_identity(nc, ident)
        nc.gpsimd.memset(cmask, 0.0)
        nc.gpsimd.affine_select(
            out=cmask, in_=cmask, compare_op=mybir.AluOpType.is_ge,
            fill=MASK_NEG, base=0, pattern=[[1, 128]], channel_multiplier=-1,
        )
        nc.vector.memset(ones_sq, 1.0)
        nc.vector.tensor_copy(out=ones2_8[:, 0, :], in_=ones_sq)
        nc.vector.tensor_copy(out=ones2_8[:, 1, :], in_=ones_sq)
        nc.vector.memset(ebias, EXP_BIAS)

        # HAM pre-warm: dummy matmuls covering the initial DMA ramp, so real
        # matmuls run at 2.4 GHz from the start with no re-throttle.
        for w in range(2):
            warm = ps_g.tile([128, 128], F32, tag="g", name=f"warm{w}")
            for _ in range(20):
                nc.tensor.matmul(warm, ones_sq, ones_sq, start=True, stop=True)

        # RoPE is software-pipelined: start_rope issues the half-swap DMAs,
        # finish_rope (called one proj pass later, when the DMA has long
        # landed) does the cos/sin muls.  This keeps the DVE from blocking on
        # DMA latency, so the next pass's PSUM-evac copy isn't queued behind a
        # stalled mul and the PE never waits on a ps_g bank.
        rope_pend = []

        def start_rope(tgt, ti, t8=None):
            sw = rope.tile([128, TQ], F16, tag="swap")
            nc.sync.dma_start(out=sw[0:64, :], in_=tgt[64:128, :])
            nc.sync.dma_start(out=sw[64:128, :], in_=tgt[0:64, :])
            rope_pend.append((tgt, sw, ti, t8))

        def finish_rope(force=False):
            # only pop entries whose swap-DMA has had a full pass to land,
            # unless force (phase end)
            if not rope_pend or (not force and len(rope_pend) < 2):
                return
            tgt, sw, ti, t8 = rope_pend.pop(0)
            tmp = rope.tile([128, TQ], F16, tag="tmp")
            nc.vector.tensor_mul(tmp, tgt, cos_sb[:, ti:ti + TQ])
            nc.vector.tensor_mul(sw, sw, sin_sb[:, ti:ti + TQ])
            nc.vector.tensor_add(tgt, tmp, sw)
            if t8 is not None:
                nc.vector.tensor_scalar_mul(t8, tgt, 16.0)

        def proj_pass(i, w_sb, col0, ncol, kind, h=None):
            """One projection output over all 16 c-chunks into 1 PSUM bank.
            q/k passes for chunks >=1 run in fp8 DoubleRow (2 contraction
            tiles per pass, 2x PE throughput); every row they produce is
            consumed with >=512 softmax terms so the quantization noise
            averages out.  fp8 x is pre-scaled by 4 and w by 256, undone at
            evac."""
            ti = TQ * i
            acc = ps_g.tile([128, TQ], F32, tag="g")
            fp8 = kind in ("k", "q") and i >= 1
            if fp8:
                w8 = wk8_sb if kind == "k" else wq8_sb
                for kk in range(0, CK, 2):
                    nc.tensor.matmul(acc, w8[:, kk:kk + 2, col0:col0 + ncol],
                                     x8_t[i][:, kk:kk + 2, :],
                                     start=(kk == 0), stop=(kk == CK - 2),
                                     perf_mode=DR)
            else:
                for kk in range(CK):
                    nc.tensor.matmul(acc, w_sb[:, kk, col0:col0 + ncol],
                                     x_t[i][:, kk, :],
                                     start=(kk == 0), stop=(kk == CK - 1))

            def evac(tgt):
                if fp8:
                    # scalar engine is idle in standalone proj phases, and
                    # fp8 passes are short enough that the DVE (evac + rope)
                    # would otherwise become the pipeline limiter
                    nc.scalar.activation(tgt, acc, AF.Copy,
                                         scale=PROJ_UNSCALE)
                else:
                    nc.vector.tensor_copy(out=tgt, in_=acc)

            if kind == "k":
                evac(kT_sb[:, ti:ti + TQ])
                start_rope(kT_sb[:, ti:ti + TQ], ti)
                finish_rope()
                return None
            if kind == "q":
                evac(qT_sb[:, h, ti:ti + TQ])
                start_rope(qT_sb[:, h, ti:ti + TQ], ti)
                finish_rope()
                return None
            vt = vtsb.tile([128, TQ], F32, tag="vt")
            nc.vector.tensor_copy(out=vt, in_=acc)
            finish_rope()
            return vt

        def drain_rope():
            while rope_pend:
                finish_rope(force=True)

        def vts(i, vt):
            """V^T -> natural [s, dv] blocks via PE transpose."""
            for jj in range(TQ // 128):
                vt_ps = ps_g.tile([128, 128], F32, tag="g")
                nc.tensor.transpose(vt_ps, vt[:, 128 * jj:128 * (jj + 1)], ident)
                nc.vector.tensor_copy(out=v_sb[:, 4 * i + jj, :], in_=vt_ps)

        def outproj_units(i, fine_dma=False):
            """16 co-block emitters for output projection of t-chunk i;
            used as PE gap-filler inside the next chunk's attention.
            fine_dma: store per-co (smaller DMAs) so the final store after
            the last matmul is short — use for the last chunk's tail."""
            ti = TQ * i
            grp = [None]

            def unit(co):
                def emit():
                    if co % 4 == 0:
                        grp[0] = outsb.tile([128, 4, TQ], F16, tag="ot",
                                            bufs=3, name=f"osb{i}_{co // 4}")
                    osb = grp[0]
                    ot = ps_g.tile([128, TQ], F32, tag="g")
                    for h in range(QH):
                        nc.tensor.matmul(ot, wo_sb[:, co, HD * h:HD * (h + 1)],
                                         oT_sb[:, h, ti:ti + TQ],
                                         start=(h == 0), stop=(h == QH - 1))
                    nc.vector.tensor_copy(out=osb[:, co % 4, :], in_=ot)
                    if fine_dma:
                        nc.sync.dma_start(
                            out=outX[i, :, co:co + 1, :],
                            in_=osb[:, co % 4:co % 4 + 1, :])
                    elif co % 4 == 3:  # store per 4-co group
                        a = co // 4
                        nc.sync.dma_start(out=outX[i, :, 4 * a:4 * a + 4, :],
                                          in_=osb)
                return emit
            return [unit(co) for co in range(CK)]

        def proj_units(i, passes):
            """q/k projection passes (fp8 DoubleRow, for chunks >=1) split
            into 2-matmul chain steps, usable as attention gap-filler.
            passes: list of (kind, h)."""
            assert i >= 1
            units = []
            pend_fin = False
            for kind, h in passes:
                w8, col0 = (wk8_sb, 0) if kind == "k" else (wq8_sb, HD * h)
                acc = ps_g.tile([128, TQ], F32, tag="g",
                                name=f"acc{i}_{kind}{h}")

                def step(acc, w8, col0, g0):
                    def emit():
                        for kk in range(g0, g0 + 4, 2):
                            nc.tensor.matmul(
                                acc, w8[:, kk:kk + 2, col0:col0 + HD],
                                x8_t[i][:, kk:kk + 2, :],
                                start=(kk == 0), stop=(kk == CK - 2),
                                perf_mode=DR)
                    return emit

                def evac(acc, kind, h, ti):
                    def emit():
                        if kind == "k":
                            tgt = kT_sb[:, ti:ti + TQ]
                        else:
                            tgt = qT_sb[:, h, ti:ti + TQ]
                        nc.vector.tensor_scalar_mul(tgt, acc, PROJ_UNSCALE)
                        start_rope(tgt, ti)
                    return emit

                for g0 in range(0, CK, 4):
                    units.append(step(acc, w8, col0, g0))
                    if pend_fin and g0 == 0:
                        # finish the PREVIOUS pass's rope one step into this
                        # pass, giving its swap-DMA time to land
                        units.append(lambda: finish_rope(force=True))
                        pend_fin = False
                units.append(evac(acc, kind, h, TQ * i))
                pend_fin = True
            if pend_fin:
                units.append(lambda: finish_rope(force=True))
            return units

        def attn_chunk(i, fillers, holdback=0):
            """Attention for t-chunk i: flat pair-stream over (head, pair)
            with one-item lookahead so PE rarely waits on exp; `fillers`
            (outproj co-blocks or next-chunk proj steps) are interleaved
            evenly to cover exp latency with useful matmuls.  The last
            `holdback` fillers are reserved until after the final flush so
            the PE has work during the last flush->reciprocal->mul chain
            (only matters when the NEXT phase depends on this chunk's oT)."""
            ti = TQ * i
            nj = 4 * (i + 1)
            npair = nj // 2
            n_items = QH * npair
            fill_idx = 0
            n_distr = len(fillers) - holdback

            def blk(j):
                t0 = max(ti, 128 * j)
                return t0, TQ * (i + 1) - t0, t0 - ti  # t0, N, c0

            acc_t = {}  # h -> (den, o_ps)

            def flush(h, p, pp, dr_info, blocks):
                if p == 0:
                    den = ps_d.tile([128, TQ], F32, tag="d",
                                    name=f"den{i}_{h}")
                    o_ps = ps_o.tile([128, TQ], F32, tag="o",
                                     name=f"o{i}_{h}")
                    acc_t[h] = (den, o_ps)
                den, o_ps = acc_t[h]
                first, last = (p == 0), (p == npair - 1)
                if dr_info is not None:
                    # fp8 DoubleRow den: one matmul sums both 128-blocks at
                    # once over their common columns (den err averages out
                    # over the >=512 softmax terms chunk>=1 rows have);
                    # diagonal pairs add an f16 matmul for block j0's
                    # leading columns
                    pp8, n1, dstc, c0l = dr_info
                    if n1 < TQ:
                        nc.tensor.matmul(den[:, c0l:c0l + 128], ones_sq,
                                         pp[:, 0:128],
                                         start=first, stop=False)
                        nc.tensor.matmul(den[:, dstc:dstc + n1], ones2_8,
                                         pp8[:, :, 0:n1],
                                         start=False, stop=last,
                                         perf_mode=DR)
                    else:
                        nc.tensor.matmul(den[:, 0:TQ], ones2_8, pp8,
                                         start=first, stop=False,
                                         perf_mode=DR)
                else:
                    for bi, (j, loc, N, c0) in enumerate(blocks):
                        st = first and bi == 0
                        sp = last and bi == len(blocks) - 1
                        nc.tensor.matmul(den[:, c0:c0 + N], ones_sq,
                                         pp[:, loc:loc + N], start=st, stop=sp)
                for bi, (j, loc, N, c0) in enumerate(blocks):
                    st = first and bi == 0
                    sp = last and bi == len(blocks) - 1
                    nc.tensor.matmul(o_ps[:, c0:c0 + N], v_sb[:, j, :],
                                     pp[:, loc:loc + N], start=st, stop=sp)
                if last:
                    inv = invp.tile([128, TQ], F32, tag="inv")
                    nc.vector.reciprocal_approx_fast(out=inv, in_=den)
                    nc.vector.tensor_mul(oT_sb[:, h, ti:ti + TQ], o_ps, inv)

            pend = None
            n = 0
            for h in range(QH):
                for p in range(npair):
                    j0, j1 = 2 * p, 2 * p + 1
                    t0a, N0, c0a = blk(j0)
                    t0b, N1, c0b = blk(j1)
                    sp_t = ps_s.tile([128, 2 * TQ], F32, tag="s")
                    nc.tensor.matmul(sp_t[:, 0:N0],
                                     kT_sb[:, 128 * j0:128 * (j0 + 1)],
                                     qT_sb[:, h, t0a:t0a + N0],
                                     start=True, stop=True)
                    nc.tensor.matmul(sp_t[:, TQ:TQ + N1],
                                     kT_sb[:, 128 * j1:128 * (j1 + 1)],
                                     qT_sb[:, h, t0b:t0b + N1],
                                     start=True, stop=True)
                    if j0 >= 4 * i:  # diagonal blocks: causal mask
                        nc.vector.tensor_add(sp_t[:, 0:128],
                                             sp_t[:, 0:128], cmask)
                    if j1 >= 4 * i:
                        nc.vector.tensor_add(sp_t[:, TQ:TQ + 128],
                                             sp_t[:, TQ:TQ + 128], cmask)
                    pp = ppool.tile([128, 2 * TQ], F16, tag="p")
                    ncols = TQ + N1
                    nc.scalar.activation(pp[:, :ncols], sp_t[:, :ncols],
                                         AF.Exp, scale=SCALE, bias=ebias)
                    dr_info = None
                    if i >= 1:  # fp8 copy for the DoubleRow den (on the
                        # otherwise-idle Pool engine)
                        pp8 = ppool.tile([128, 2, TQ], FP8, tag="p8")
                        if j1 < 4 * i:      # off-diagonal: full width
                            nc.gpsimd.tensor_copy(out=pp8[:, 0, :],
                                                  in_=pp[:, 0:TQ])
                            nc.gpsimd.tensor_copy(out=pp8[:, 1, :],
                                                  in_=pp[:, TQ:2 * TQ])
                            dr_info = (pp8, TQ, 0, 0)
                        else:               # diagonal: common N1 columns
                            off = c0b - c0a
                            nc.gpsimd.tensor_copy(out=pp8[:, 0, 0:N1],
                                                  in_=pp[:, off:off + N1])
                            nc.gpsimd.tensor_copy(out=pp8[:, 1, 0:N1],
                                                  in_=pp[:, TQ:TQ + N1])
                            dr_info = (pp8, N1, c0b, c0a)
                    if pend is not None:
                        flush(*pend)
                    pend = (h, p, pp, dr_info,
                            [(j0, 0, N0, c0a), (j1, TQ, N1, c0b)])
                    n += 1
                    while fill_idx * n_items < n * n_distr:
                        fillers[fill_idx]()
                        fill_idx += 1
            # run most held-back fillers BEFORE the final flush (they cover
            # the last exp's latency), keep one for the reciprocal+mul tail
            while fill_idx < len(fillers) - 1 and holdback > 0:
                fillers[fill_idx]()
                fill_idx += 1
            flush(*pend)
            while fill_idx < len(fillers):
                fillers[fill_idx]()
                fill_idx += 1

        def full_proj(i):
            """All projections for chunk i, V-transposes mid-way so they
            don't queue behind all the RoPE work on the DVE."""
            proj_pass(i, wk_sb, 0, HD, "k")
            proj_pass(i, wq_sb, 0, HD, "q", h=0)
            proj_pass(i, wq_sb, HD, HD, "q", h=1)
            vt = proj_pass(i, wv_sb, 0, HD, "v")
            vts(i, vt)
            proj_pass(i, wq_sb, 2 * HD, HD, "q", h=2)
            proj_pass(i, wq_sb, 3 * HD, HD, "q", h=3)
            drain_rope()

        # ======== fused pipeline ========
        # chunk 0 projections, then attn(0) filled with proj(1) k/q0 steps,
        # then the rest of proj(1), then attn(i) filled with outproj(i-1).
        full_proj(0)
        attn_chunk(0, proj_units(1, [("k", None), ("q", 0)]))
        proj_pass(1, wq_sb, HD, HD, "q", h=1)
        vt = proj_pass(1, wv_sb, 0, HD, "v")
        vts(1, vt)
        proj_pass(1, wq_sb, 2 * HD, HD, "q", h=2)
        proj_pass(1, wq_sb, 3 * HD, HD, "q", h=3)
        drain_rope()
        load_x8(2)
        load_x(2)
        attn_chunk(1, outproj_units(0))
        full_proj(2)
        load_x8(3)
        load_x(3)
        attn_chunk(2, outproj_units(1))
        full_proj(3)
        attn_chunk(3, outproj_units(2), holdback=5)
        for u in outproj_units(3, fine_dma=True):
            u()
        # tail warmers: keep the PE active while the last evac copies/DMAs
        # drain, so HAM doesn't down-clock and stretch the drain sequence
        tailw = ps_g.tile([128, 128], F32, tag="g", name="tailw")
        for _ in range(48):
            nc.tensor.matmul(tailw, ones_sq, ones_sq, start=True, stop=True)


_PERM = np.concatenate([np.arange(0, HD, 2), np.arange(1, HD, 2)])

PROFILE = False
LAST_EXEC_NS = None
LAST_RESULTS = None


def kernel(x, freqs_cos, freqs_sin, wq, wk, wv, wo):
    global LAST_EXEC_NS, LAST_RESULTS
    if "nc" not in _CACHE:
        _CACHE["nc"] = _build_nc()
    nc = _CACHE["nc"]

    x = np.asarray(x, dtype=np.float32)
    fc = np.asarray(freqs_cos, dtype=np.float32)
    fs = np.asarray(freqs_sin, dtype=np.float32)
    wq = np.asarray(wq, dtype=np.float32)
    wk = np.asarray(wk, dtype=np.float32)
    wv = np.asarray(wv, dtype=np.float32)
    wo = np.asarray(wo, dtype=np.float32)

    cosT = fc.T                                   # [64, T]
    sinT = fs.T
    cosH = np.ascontiguousarray(
        np.concatenate([cosT, cosT], axis=0).astype(np.float16))   # [128, T]
    sinH = np.ascontiguousarray(
        np.concatenate([-sinT, sinT], axis=0).astype(np.float16))

    import ml_dtypes
    E4 = ml_dtypes.float8_e4m3

    in_maps = []
    for core in range(8):
        b, g = core // 4, core % 4
        xT32 = x[b].T                                         # [C, T] f32
        xT = xT32.astype(np.float16)
        # [C, T] -> [NT, 128(p), CK(k), TQ]: xH[i, p, k, t] = xT[128k+p, 512i+t]
        xH = np.ascontiguousarray(
            xT.reshape(CK, 128, NT, TQ).transpose(2, 1, 0, 3))
        # fp8 copy, pre-scaled by 4 (see kernel comment)
        x8H = np.ascontiguousarray(
            (xT32 * 4.0).reshape(CK, 128, NT, TQ).transpose(2, 1, 0, 3)
        ).astype(E4)
        wq_g = wq[512 * g:512 * (g + 1)].reshape(QH, HD, C)[:, _PERM, :]
        wqT32 = wq_g.reshape(QH * HD, C).T                    # [C, 512] f32
        wqT = wqT32.astype(np.float16)
        wqH = np.ascontiguousarray(
            wqT.reshape(CK, 128, QH * HD).transpose(1, 0, 2))  # [128, CK, 512]
        wq8H = np.ascontiguousarray(
            (wqT32 * 256.0).reshape(CK, 128, QH * HD).transpose(1, 0, 2)
        ).astype(E4)
        wkT32 = wk[HD * g:HD * (g + 1)][_PERM].T              # [C, 128] f32
        wkT = wkT32.astype(np.float16)
        wkH = np.ascontiguousarray(wkT.reshape(CK, 128, HD).transpose(1, 0, 2))
        wk8H = np.ascontiguousarray(
            (wkT32 * 256.0).reshape(CK, 128, HD).transpose(1, 0, 2)
        ).astype(E4)
        wvT = wv[HD * g:HD * (g + 1)].T.astype(np.float16)
        wvH = np.ascontiguousarray(wvT.reshape(CK, 128, HD).transpose(1, 0, 2))
        wo_g = wo[:, 512 * g:512 * (g + 1)]                   # [C, 512]
        # woH[p, co, 128h+d] = wo[128co+d, 512g+128h+p]
        woH = np.ascontiguousarray(
            wo_g.reshape(CK, 128, QH, 128).transpose(3, 0, 2, 1)
        ).astype(np.float16).reshape(128, CK, QH * 128)
        in_maps.append({
            "xH": xH, "wqH": wqH, "wkH": wkH, "wvH": wvH, "woH": woH,
            "x8H": x8H, "wq8H": wq8H, "wk8H": wk8H,
            "cosH": cosH, "sinH": sinH,
        })

    res = run_bass_kernel_spmd(nc, in_maps, list(range(8)), trace=PROFILE)
    LAST_EXEC_NS = res.exec_time_ns
    LAST_RESULTS = res

    out = np.empty((B, T, C), dtype=np.float32)
    for b in range(B):
        acc = res.results[4 * b]["outX"].astype(np.float32)
        for g in range(1, 4):
            acc = acc + res.results[4 * b + g]["outX"]
        # outX[i, d?, co, t]: out[b][512i+t, 128co+d] = outX[i, d, co, t]
        out[b] = acc.transpose(0, 3, 2, 1).reshape(T, C)
    return out



# revision 53
# speedup vs baseline: 1.4691x; 1.4691x over previous
"""Causal self-attention (GQA + RoPE) Trainium2 Bass kernel, 8 NeuronCores.

Problem: B=2, T=2048, C=2048, n_head=16, n_kv_head=4, head_dim=128.

Sharding: 2-way batch DP x 4-way head TP. Core c = 4*b + g handles batch b,
kv head g, q heads [4g, 4g+4). wq/wk/wv column-sharded per head group, wo
row-sharded; per-core partial outputs are summed on the host (the gather /
unshard step), so no on-device collective is needed.

v3: fully fused pipeline. Per 512-col t-chunk i the PE emission order is
  proj(i) -> V-transpose(i) -> outproj(i-1) -> attention(i)
so the tensor engine never crosses a phase barrier (keeps HAM warm).
RoPE is software-pipelined (swap-DMA issued at evac, cos/sin muls one pass
later) so the DVE never blocks on DMA latency and PSUM banks recycle
promptly; attn(3) holds back 3 outproj(2) fillers to cover the final
flush->reciprocal->mul chain; the last chunk's stores are per-co DMAs.
Projection runs one output at a time (k, q0..q3, v: 16-chunk accumulation
chains in a single PSUM bank each) so projections need only the 2 shared
"generic" PSUM banks; attention uses 2-bank score pairs (one exp per block
pair), accumulating den (ones-matmul) and O over s-blocks; softmax denom
reciprocal via the fast approx DVE op. All DRAM inputs are host-pre-tiled
to match SBUF layouts so every DMA is contiguous; outputs are fp16
partials summed on the host.
"""

import sys

sys.path.insert(0, "/opt/trn_rl_repo")

import numpy as np

import concourse.bass as bass
import concourse.mybir as mybir
import concourse.tile as tile
from concourse import bacc
from concourse.bass_utils import run_bass_kernel_spmd
from concourse.masks import make_identity

F32 = mybir.dt.float32
F16 = mybir.dt.float16
FP8 = mybir.dt.float8e4
AF = mybir.ActivationFunctionType
DR = mybir.MatmulPerfMode.DoubleRow
DP = mybir.MatmulPerfMode.DoublePixel

B, T, C = 2, 2048, 2048
N_HEAD, N_KV_HEAD = 16, 4
HD = 128                 # head dim
QH = 4                   # q heads per core
TQ = 512                 # t-chunk
NT = T // TQ             # 4 t-chunks
CK = C // 128            # 16 contraction chunks of 128
SCALE = 1.0 / float(np.sqrt(HD))
# exp is computed as exp(score*SCALE - ln8) = exp(score*SCALE)/8; the /8
# cancels in O/den but keeps the fp8e4 den copy far from TRN-fp8's +-240
# max (values >=248 cast to inf -> NaN through the reciprocal).
EXP_BIAS = -float(np.log(8.0))
MASK_NEG = -1e30

_CACHE = {}


def _build_nc():
    nc = bacc.Bacc("TRN2", target_bir_lowering=False, debug=False, num_devices=8)

    # All inputs pre-tiled on host so DRAM layout == SBUF layout.
    xH = nc.dram_tensor("xH", [NT, 128, CK, TQ], F16, kind="ExternalInput").ap()
    wqH = nc.dram_tensor("wqH", [128, CK, QH * HD], F16, kind="ExternalInput").ap()
    wkH = nc.dram_tensor("wkH", [128, CK, HD], F16, kind="ExternalInput").ap()
    wvH = nc.dram_tensor("wvH", [128, CK, HD], F16, kind="ExternalInput").ap()
    woH = nc.dram_tensor("woH", [128, CK, QH * HD], F16, kind="ExternalInput").ap()
    cosH = nc.dram_tensor("cosH", [HD, T], F16, kind="ExternalInput").ap()
    sinH = nc.dram_tensor("sinH", [HD, T], F16, kind="ExternalInput").ap()
    # fp8 copies for the chunk>=1 q/k projections (x pre-scaled by 4, w by
    # 256 so weights sit in fp8e4's normal range; evac divides by 1024)
    x8H = nc.dram_tensor("x8H", [NT, 128, CK, TQ], FP8, kind="ExternalInput").ap()
    wq8H = nc.dram_tensor("wq8H", [128, CK, QH * HD], FP8,
                          kind="ExternalInput").ap()
    wk8H = nc.dram_tensor("wk8H", [128, CK, HD], FP8, kind="ExternalInput").ap()
    outX = nc.dram_tensor("outX", [NT, 128, CK, TQ], F16, kind="ExternalOutput").ap()

    with tile.TileContext(nc) as tc:
        _emit(nc, tc, xH, wqH, wkH, wvH, woH, cosH, sinH,
              x8H, wq8H, wk8H, outX)

    nc.compile()
    return nc


PROJ_UNSCALE = 1.0 / 1024.0


def _emit(nc, tc, xH, wqH, wkH, wvH, woH, cosH, sinH, x8H, wq8H, wk8H, outX):
    import contextlib

    ctx = contextlib.ExitStack()
    with ctx:
        singles = ctx.enter_context(tc.tile_pool(name="singles", bufs=1))

        # ---- resident tiles ----
        wq_sb = singles.tile([128, CK, QH * HD], F16)
        wk_sb = singles.tile([128, CK, HD], F16)
        wv_sb = singles.tile([128, CK, HD], F16)
        wo_sb = singles.tile([128, CK, QH * HD], F16)
        wq8_sb = singles.tile([128, CK, QH * HD], FP8)
        wk8_sb = singles.tile([128, CK, HD], FP8)
        cos_sb = singles.tile([HD, T], F16)
        sin_sb = singles.tile([HD, T], F16)

        qT_sb = singles.tile([128, QH, T], F16)    # per head [dq, t], RoPE'd
        kT_sb = singles.tile([128, T], F16)        # [dk, t], RoPE'd
        v_sb = singles.tile([128, CK, HD], F16)    # [s in blk, (blk, dv)]
        oT_sb = singles.tile([128, QH, T], F16)    # per head [dv, t] normalized

        ident = singles.tile([128, 128], F32)
        cmask = singles.tile([128, 128], F32)
        ones_sq = singles.tile([128, 128], F16)
        ones2_8 = singles.tile([128, 2, 128], FP8)
        ebias = singles.tile([128, 1], F32)

        # ---- pools ----
        xpool = ctx.enter_context(tc.tile_pool(name="xpool", bufs=2))
        x8pool = ctx.enter_context(tc.tile_pool(name="x8pool", bufs=2))
        ppool = ctx.enter_context(tc.tile_pool(name="ppool", bufs=3))
        vtsb = ctx.enter_context(tc.tile_pool(name="vtsb", bufs=2))
        rope = ctx.enter_context(tc.tile_pool(name="rope", bufs=3))
        invp = ctx.enter_context(tc.tile_pool(name="invp", bufs=2))
        outsb = ctx.enter_context(tc.tile_pool(name="outsb", bufs=2))
        ps_s = ctx.enter_context(tc.tile_pool(name="ps_s", bufs=2, space="PSUM"))
        ps_d = ctx.enter_context(tc.tile_pool(name="ps_d", bufs=1, space="PSUM"))
        ps_o = ctx.enter_context(tc.tile_pool(name="ps_o", bufs=1, space="PSUM"))
        ps_g = ctx.enter_context(tc.tile_pool(name="ps_g", bufs=2, space="PSUM"))

        # ---- startup DMAs, criticality-ordered ----
        x_t = [None] * NT
        x8_t = [None] * NT

        def load_x(i):
            x_t[i] = xpool.tile([128, CK, TQ], F16, tag="x", name=f"x{i}")
            for a in range(2):
                nc.sync.dma_start(out=x_t[i][:, 8 * a:8 * a + 8, :],
                                  in_=xH[i, :, 8 * a:8 * a + 8, :])

        def load_x8(i):
            x8_t[i] = x8pool.tile([128, CK, TQ], FP8, tag="x8", name=f"x8_{i}")
            for a in range(2):
                nc.sync.dma_start(out=x8_t[i][:, 8 * a:8 * a + 8, :],
                                  in_=x8H[i, :, 8 * a:8 * a + 8, :])

        # Startup DMAs: each dma_start costs ~600ns of ISSUE time on its
        # engine (transfers then run async), and nothing issues before the
        # engine programs load (~7us).  So: few, coarse pieces, split across
        # BOTH hw-DGE engines (sync + scalar) so the two issue queues run in
        # parallel.  sync gets the critical-path x0/wk; scalar gets wq and
        # the rest.
        nc.sync.dma_start(out=wk_sb, in_=wkH)
        x_t[0] = xpool.tile([128, CK, TQ], F16, tag="x", name="x0")
        for a in range(4):
            nc.sync.dma_start(out=x_t[0][:, 4 * a:4 * a + 4, :],
                              in_=xH[0, :, 4 * a:4 * a + 4, :])
        for a in range(2):
            nc.scalar.dma_start(out=wq_sb[:, 8 * a:8 * a + 8, :],
                                in_=wqH[:, 8 * a:8 * a + 8, :])
        nc.scalar.dma_start(out=cos_sb, in_=cosH)
        nc.scalar.dma_start(out=sin_sb, in_=sinH)
        nc.scalar.dma_start(out=wv_sb, in_=wvH)
        nc.scalar.dma_start(out=wq8_sb, in_=wq8H)
        nc.scalar.dma_start(out=wk8_sb, in_=wk8H)
        load_x8(1)  # needed by attn(0)'s chunk-1 k/q0 fillers
        load_x(1)
        nc.sync.dma_start(out=wo_sb, in_=woH)

        make_identity(nc, ident)
        nc.gpsimd.memset(cmask, 0.0)
        nc.gpsimd.affine_select(
            out=cmask, in_=cmask, compare_op=mybir.AluOpType.is_ge,
            fill=MASK_NEG, base=0, pattern=[[1, 128]], channel_multiplier=-1,
        )
        nc.vector.memset(ones_sq, 1.0)
        nc.vector.tensor_copy(out=ones2_8[:, 0, :], in_=ones_sq)
        nc.vector.tensor_copy(out=ones2_8[:, 1, :], in_=ones_sq)
        nc.vector.memset(ebias, EXP_BIAS)

        # HAM pre-warm: dummy matmuls covering the initial DMA ramp, so real
        # matmuls run at 2.4 GHz from the start with no re-throttle.
        for w in range(2):
            warm = ps_g.tile([128, 128], F32, tag="g", name=f"warm{w}")
            for _ in range(20):
                nc.tensor.matmul(warm, ones_sq, ones_sq, start=True, stop=True)

        # RoPE is software-pipelined: start_rope issues the half-swap DMAs,
        # finish_rope (called one proj pass later, when the DMA has long
        # landed) does the cos/sin muls.  This keeps the DVE from blocking on
        # DMA latency, so the next pass's PSUM-evac copy isn't queued behind a
        # stalled mul and the PE never waits on a ps_g bank.
        rope_pend = []

        def start_rope(tgt, ti, t8=None):
            sw = rope.tile([128, TQ], F16, tag="swap")
            nc.sync.dma_start(out=sw[0:64, :], in_=tgt[64:128, :])
            nc.sync.dma_start(out=sw[64:128, :], in_=tgt[0:64, :])
            rope_pend.append((tgt, sw, ti, t8))

        def finish_rope(force=False):
            # only pop entries whose swap-DMA has had a full pass to land,
            # unless force (phase end)
            if not rope_pend or (not force and len(rope_pend) < 2):
                return
            tgt, sw, ti, t8 = rope_pend.pop(0)
            tmp = rope.tile([128, TQ], F16, tag="tmp")
            nc.vector.tensor_mul(tmp, tgt, cos_sb[:, ti:ti + TQ])
            nc.vector.tensor_mul(sw, sw, sin_sb[:, ti:ti + TQ])
            nc.vector.tensor_add(tgt, tmp, sw)
            if t8 is not None:
                nc.vector.tensor_scalar_mul(t8, tgt, 16.0)

        def proj_pass(i, w_sb, col0, ncol, kind, h=None):
            """One projection output over all 16 c-chunks into 1 PSUM bank.
            q/k passes for chunks >=1 run in fp8 DoubleRow (2 contraction
            tiles per pass, 2x PE throughput); every row they produce is
            consumed with >=512 softmax terms so the quantization noise
            averages out.  fp8 x is pre-scaled by 4 and w by 256, undone at
            evac."""
            ti = TQ * i
            acc = ps_g.tile([128, TQ], F32, tag="g")
            fp8 = kind in ("k", "q") and i >= 1
            if fp8:
                w8 = wk8_sb if kind == "k" else wq8_sb
                for kk in range(0, CK, 2):
                    nc.tensor.matmul(acc, w8[:, kk:kk + 2, col0:col0 + ncol],
                                     x8_t[i][:, kk:kk + 2, :],
                                     start=(kk == 0), stop=(kk == CK - 2),
                                     perf_mode=DR)
            else:
                for kk in range(CK):
                    nc.tensor.matmul(acc, w_sb[:, kk, col0:col0 + ncol],
                                     x_t[i][:, kk, :],
                                     start=(kk == 0), stop=(kk == CK - 1))

            def evac(tgt):
                if fp8:
                    # scalar engine is idle in standalone proj phases, and
                    # fp8 passes are short enough that the DVE (evac + rope)
                    # would otherwise become the pipeline limiter
                    nc.scalar.activation(tgt, acc, AF.Copy,
                                         scale=PROJ_UNSCALE)
                else:
                    nc.vector.tensor_copy(out=tgt, in_=acc)

            if kind == "k":
                evac(kT_sb[:, ti:ti + TQ])
                start_rope(kT_sb[:, ti:ti + TQ], ti)
                finish_rope()
                return None
            if kind == "q":
                evac(qT_sb[:, h, ti:ti + TQ])
                start_rope(qT_sb[:, h, ti:ti + TQ], ti)
                finish_rope()
                return None
            vt = vtsb.tile([128, TQ], F32, tag="vt")
            nc.vector.tensor_copy(out=vt, in_=acc)
            finish_rope()
            return vt

        def drain_rope():
            while rope_pend:
                finish_rope(force=True)

        def vts(i, vt):
            """V^T -> natural [s, dv] blocks via PE transpose."""
            for jj in range(TQ // 128):
                vt_ps = ps_g.tile([128, 128], F32, tag="g")
                nc.tensor.transpose(vt_ps, vt[:, 128 * jj:128 * (jj + 1)], ident)
                nc.vector.tensor_copy(out=v_sb[:, 4 * i + jj, :], in_=vt_ps)

        def outproj_units(i, fine_dma=False):
            """16 co-block emitters for output projection of t-chunk i;
            used as PE gap-filler inside the next chunk's attention.
            fine_dma: store per-co (smaller DMAs) so the final store after
            the last matmul is short — use for the last chunk's tail."""
            ti = TQ * i
            grp = [None]

            def unit(co):
                def emit():
                    if co % 4 == 0:
                        grp[0] = outsb.tile([128, 4, TQ], F16, tag="ot",
                                            bufs=3, name=f"osb{i}_{co // 4}")
                    osb = grp[0]
                    ot = ps_g.tile([128, TQ], F32, tag="g")
                    for h in range(QH):
                        nc.tensor.matmul(ot, wo_sb[:, co, HD * h:HD * (h + 1)],
                                         oT_sb[:, h, ti:ti + TQ],
                                         start=(h == 0), stop=(h == QH - 1))
                    nc.vector.tensor_copy(out=osb[:, co % 4, :], in_=ot)
                    if fine_dma:
                        nc.sync.dma_start(
                            out=outX[i, :, co:co + 1, :],
                            in_=osb[:, co % 4:co % 4 + 1, :])
                    elif co % 4 == 3:  # store per 4-co group
                        a = co // 4
                        nc.sync.dma_start(out=outX[i, :, 4 * a:4 * a + 4, :],
                                          in_=osb)
                return emit
            return [unit(co) for co in range(CK)]

        def proj_units(i, passes):
            """q/k projection passes (fp8 DoubleRow, for chunks >=1) split
            into 2-matmul chain steps, usable as attention gap-filler.
            passes: list of (kind, h)."""
            assert i >= 1
            units = []
            pend_fin = False
            for kind, h in passes:
                w8, col0 = (wk8_sb, 0) if kind == "k" else (wq8_sb, HD * h)
                acc = ps_g.tile([128, TQ], F32, tag="g",
                                name=f"acc{i}_{kind}{h}")

                def step(acc, w8, col0, g0):
                    def emit():
                        for kk in range(g0, g0 + 4, 2):
                            nc.tensor.matmul(
                                acc, w8[:, kk:kk + 2, col0:col0 + HD],
                                x8_t[i][:, kk:kk + 2, :],
                                start=(kk == 0), stop=(kk == CK - 2),
                                perf_mode=DR)
                    return emit

                def evac(acc, kind, h, ti):
                    def emit():
                        if kind == "k":
                            tgt = kT_sb[:, ti:ti + TQ]
                        else:
                            tgt = qT_sb[:, h, ti:ti + TQ]
                        nc.vector.tensor_scalar_mul(tgt, acc, PROJ_UNSCALE)
                        start_rope(tgt, ti)
                    return emit

                for g0 in range(0, CK, 4):
                    units.append(step(acc, w8, col0, g0))
                    if pend_fin and g0 == 0:
                        # finish the PREVIOUS pass's rope one step into this
                        # pass, giving its swap-DMA time to land
                        units.append(lambda: finish_rope(force=True))
                        pend_fin = False
                units.append(evac(acc, kind, h, TQ * i))
                pend_fin = True
            if pend_fin:
                units.append(lambda: finish_rope(force=True))
            return units

        def attn_chunk(i, fillers, holdback=0):
            """Attention for t-chunk i: flat pair-stream over (head, pair)
            with one-item lookahead so PE rarely waits on exp; `fillers`
            (outproj co-blocks or next-chunk proj steps) are interleaved
            evenly to cover exp latency with useful matmuls.  The last
            `holdback` fillers are reserved until after the final flush so
            the PE has work during the last flush->reciprocal->mul chain
            (only matters when the NEXT phase depends on this chunk's oT)."""
            ti = TQ * i
            nj = 4 * (i + 1)
            npair = nj // 2
            n_items = QH * npair
            fill_idx = 0
            n_distr = len(fillers) - holdback

            def blk(j):
                t0 = max(ti, 128 * j)
                return t0, TQ * (i + 1) - t0, t0 - ti  # t0, N, c0

            acc_t = {}  # h -> (den, o_ps)

            def flush(h, p, pp, dr_info, blocks):
                if p == 0:
                    den = ps_d.tile([128, TQ], F32, tag="d",
                                    name=f"den{i}_{h}")
                    o_ps = ps_o.tile([128, TQ], F32, tag="o",
                                     name=f"o{i}_{h}")
                    acc_t[h] = (den, o_ps)
                den, o_ps = acc_t[h]
                first, last = (p == 0), (p == npair - 1)
                if dr_info is not None:
                    # fp8 DoubleRow den: one matmul sums both 128-blocks at
                    # once over their common columns (den err averages out
                    # over the >=512 softmax terms chunk>=1 rows have);
                    # diagonal pairs add an f16 matmul for block j0's
                    # leading columns
                    pp8, n1, dstc, c0l = dr_info
                    if n1 < TQ:
                        nc.tensor.matmul(den[:, c0l:c0l + 128], ones_sq,
                                         pp[:, 0:128],
                                         start=first, stop=False)
                        nc.tensor.matmul(den[:, dstc:dstc + n1], ones2_8,
                                         pp8[:, :, 0:n1],
                                         start=False, stop=last,
                                         perf_mode=DR)
                    else:
                        nc.tensor.matmul(den[:, 0:TQ], ones2_8, pp8,
                                         start=first, stop=False,
                                         perf_mode=DR)
                else:
                    for bi, (j, loc, N, c0) in enumerate(blocks):
                        st = first and bi == 0
                        sp = last and bi == len(blocks) - 1
                        nc.tensor.matmul(den[:, c0:c0 + N], ones_sq,
                                         pp[:, loc:loc + N], start=st, stop=sp)
                for bi, (j, loc, N, c0) in enumerate(blocks):
                    st = first and bi == 0
                    sp = last and bi == len(blocks) - 1
                    nc.tensor.matmul(o_ps[:, c0:c0 + N], v_sb[:, j, :],
                                     pp[:, loc:loc + N], start=st, stop=sp)
                if last:
                    inv = invp.tile([128, TQ], F32, tag="inv")
                    nc.vector.reciprocal_approx_fast(out=inv, in_=den)
                    nc.vector.tensor_mul(oT_sb[:, h, ti:ti + TQ], o_ps, inv)

            pend = None
            n = 0
            for h in range(QH):
                for p in range(npair):
                    j0, j1 = 2 * p, 2 * p + 1
                    t0a, N0, c0a = blk(j0)
                    t0b, N1, c0b = blk(j1)
                    sp_t = ps_s.tile([128, 2 * TQ], F32, tag="s")
                    nc.tensor.matmul(sp_t[:, 0:N0],
                                     kT_sb[:, 128 * j0:128 * (j0 + 1)],
                                     qT_sb[:, h, t0a:t0a + N0],
                                     start=True, stop=True)
                    nc.tensor.matmul(sp_t[:, TQ:TQ + N1],
                                     kT_sb[:, 128 * j1:128 * (j1 + 1)],
                                     qT_sb[:, h, t0b:t0b + N1],
                                     start=True, stop=True)
                    if j0 >= 4 * i:  # diagonal blocks: causal mask
                        nc.vector.tensor_add(sp_t[:, 0:128],
                                             sp_t[:, 0:128], cmask)
                    if j1 >= 4 * i:
                        nc.vector.tensor_add(sp_t[:, TQ:TQ + 128],
                                             sp_t[:, TQ:TQ + 128], cmask)
                    pp = ppool.tile([128, 2 * TQ], F16, tag="p")
                    ncols = TQ + N1
                    nc.scalar.activation(pp[:, :ncols], sp_t[:, :ncols],
                                         AF.Exp, scale=SCALE, bias=ebias)
                    dr_info = None
                    if i >= 1:  # fp8 copy for the DoubleRow den
                        pp8 = ppool.tile([128, 2, TQ], FP8, tag="p8")
                        if j1 < 4 * i:      # off-diagonal: full width
                            nc.vector.tensor_copy(out=pp8[:, 0, :],
                                                  in_=pp[:, 0:TQ])
                            nc.vector.tensor_copy(out=pp8[:, 1, :],
                                                  in_=pp[:, TQ:2 * TQ])
                            dr_info = (pp8, TQ, 0, 0)
                        else:               # diagonal: common N1 columns
                            off = c0b - c0a
                            nc.vector.tensor_copy(out=pp8[:, 0, 0:N1],
                                                  in_=pp[:, off:off + N1])
                            nc.vector.tensor_copy(out=pp8[:, 1, 0:N1],
                                                  in_=pp[:, TQ:TQ + N1])
                            dr_info = (pp8, N1, c0b, c0a)
                    if pend is not None:
                        flush(*pend)
                    pend = (h, p, pp, dr_info,
                            [(j0, 0, N0, c0a), (j1, TQ, N1, c0b)])
                    n += 1
                    while fill_idx * n_items < n * n_distr:
                        fillers[fill_idx]()
                        fill_idx += 1
            # run most held-back fillers BEFORE the final flush (they cover
            # the last exp's latency), keep one for the reciprocal+mul tail
            while fill_idx < len(fillers) - 1 and holdback > 0:
                fillers[fill_idx]()
                fill_idx += 1
            flush(*pend)
            while fill_idx < len(fillers):
                fillers[fill_idx]()
                fill_idx += 1

        def full_proj(i):
            """All projections for chunk i, V-transposes mid-way so they
            don't queue behind all the RoPE work on the DVE."""
            proj_pass(i, wk_sb, 0, HD, "k")
            proj_pass(i, wq_sb, 0, HD, "q", h=0)
            proj_pass(i, wq_sb, HD, HD, "q", h=1)
            vt = proj_pass(i, wv_sb, 0, HD, "v")
            vts(i, vt)
            proj_pass(i, wq_sb, 2 * HD, HD, "q", h=2)
            proj_pass(i, wq_sb, 3 * HD, HD, "q", h=3)
            drain_rope()

        # ======== fused pipeline ========
        # chunk 0 projections, then attn(0) filled with proj(1) k/q0 steps,
        # then the rest of proj(1), then attn(i) filled with outproj(i-1).
        full_proj(0)
        attn_chunk(0, proj_units(1, [("k", None), ("q", 0)]))
        proj_pass(1, wq_sb, HD, HD, "q", h=1)
        vt = proj_pass(1, wv_sb, 0, HD, "v")
        vts(1, vt)
        proj_pass(1, wq_sb, 2 * HD, HD, "q", h=2)
        proj_pass(1, wq_sb, 3 * HD, HD, "q", h=3)
        drain_rope()
        load_x8(2)
        load_x(2)
        attn_chunk(1, outproj_units(0))
        full_proj(2)
        load_x8(3)
        load_x(3)
        attn_chunk(2, outproj_units(1))
        full_proj(3)
        attn_chunk(3, outproj_units(2), holdback=5)
        for u in outproj_units(3, fine_dma=True):
            u()
        # tail warmers: keep the PE active while the last evac copies/DMAs
        # drain, so HAM doesn't down-clock and stretch the drain sequence
        tailw = ps_g.tile([128, 128], F32, tag="g", name="tailw")
        for _ in range(48):
            nc.tensor.matmul(tailw, ones_sq, ones_sq, start=True, stop=True)


_PERM = np.concatenate([np.arange(0, HD, 2), np.arange(1, HD, 2)])

PROFILE = False
LAST_EXEC_NS = None
LAST_RESULTS = None


def kernel(x, freqs_cos, freqs_sin, wq, wk, wv, wo):
    global LAST_EXEC_NS, LAST_RESULTS
    if "nc" not in _CACHE:
        _CACHE["nc"] = _build_nc()
    nc = _CACHE["nc"]

    x = np.asarray(x, dtype=np.float32)
    fc = np.asarray(freqs_cos, dtype=np.float32)
    fs = np.asarray(freqs_sin, dtype=np.float32)
    wq = np.asarray(wq, dtype=np.float32)
    wk = np.asarray(wk, dtype=np.float32)
    wv = np.asarray(wv, dtype=np.float32)
    wo = np.asarray(wo, dtype=np.float32)

    cosT = fc.T                                   # [64, T]
    sinT = fs.T
    cosH = np.ascontiguousarray(
        np.concatenate([cosT, cosT], axis=0).astype(np.float16))   # [128, T]
    sinH = np.ascontiguousarray(
        np.concatenate([-sinT, sinT], axis=0).astype(np.float16))

    import ml_dtypes
    E4 = ml_dtypes.float8_e4m3

    in_maps = []
    for core in range(8):
        b, g = core // 4, core % 4
        xT32 = x[b].T                                         # [C, T] f32
        xT = xT32.astype(np.float16)
        # [C, T] -> [NT, 128(p), CK(k), TQ]: xH[i, p, k, t] = xT[128k+p, 512i+t]
        xH = np.ascontiguousarray(
            xT.reshape(CK, 128, NT, TQ).transpose(2, 1, 0, 3))
        # fp8 copy, pre-scaled by 4 (see kernel comment)
        x8H = np.ascontiguousarray(
            (xT32 * 4.0).reshape(CK, 128, NT, TQ).transpose(2, 1, 0, 3)
        ).astype(E4)
        wq_g = wq[512 * g:512 * (g + 1)].reshape(QH, HD, C)[:, _PERM, :]
        wqT32 = wq_g.reshape(QH * HD, C).T                    # [C, 512] f32
        wqT = wqT32.astype(np.float16)
        wqH = np.ascontiguousarray(
            wqT.reshape(CK, 128, QH * HD).transpose(1, 0, 2))  # [128, CK, 512]
        wq8H = np.ascontiguousarray(
            (wqT32 * 256.0).reshape(CK, 128, QH * HD).transpose(1, 0, 2)
        ).astype(E4)
        wkT32 = wk[HD * g:HD * (g + 1)][_PERM].T              # [C, 128] f32
        wkT = wkT32.astype(np.float16)
        wkH = np.ascontiguousarray(wkT.reshape(CK, 128, HD).transpose(1, 0, 2))
        wk8H = np.ascontiguousarray(
            (wkT32 * 256.0).reshape(CK, 128, HD).transpose(1, 0, 2)
        ).astype(E4)
        wvT = wv[HD * g:HD * (g + 1)].T.astype(np.float16)
        wvH = np.ascontiguousarray(wvT.reshape(CK, 128, HD).transpose(1, 0, 2))
        wo_g = wo[:, 512 * g:512 * (g + 1)]                   # [C, 512]
        # woH[p, co, 128h+d] = wo[128co+d, 512g+128h+p]
        woH = np.ascontiguousarray(
            wo_g.reshape(CK, 128, QH, 128).transpose(3, 0, 2, 1)
        ).astype(np.float16).reshape(128, CK, QH * 128)
        in_maps.append({
            "xH": xH, "wqH": wqH, "wkH": wkH, "wvH": wvH, "woH": woH,
            "x8H": x8H, "wq8H": wq8H, "wk8H": wk8H,
            "cosH": cosH, "sinH": sinH,
        })

    res = run_bass_kernel_spmd(nc, in_maps, list(range(8)), trace=PROFILE)
    LAST_EXEC_NS = res.exec_time_ns
    LAST_RESULTS = res

    out = np.empty((B, T, C), dtype=np.float32)
    for b in range(B):
        acc = res.results[4 * b]["outX"].astype(np.float32)
        for g in range(1, 4):
            acc = acc + res.results[4 * b + g]["outX"]
        # outX[i, d?, co, t]: out[b][512i+t, 128co+d] = outX[i, d, co, t]
        out[b] = acc.transpose(0, 3, 2, 1).reshape(T, C)
    return out



# revision 54
# speedup vs baseline: 1.5060x; 1.0251x over previous
"""Causal self-attention (GQA + RoPE) Trainium2 Bass kernel, 8 NeuronCores.

Problem: B=2, T=2048, C=2048, n_head=16, n_kv_head=4, head_dim=128.

Sharding: 2-way batch DP x 4-way head TP. Core c = 4*b + g handles batch b,
kv head g, q heads [4g, 4g+4). wq/wk/wv column-sharded per head group, wo
row-sharded; per-core partial outputs are summed on the host (the gather /
unshard step), so no on-device collective is needed.

v3: fully fused pipeline. Per 512-col t-chunk i the PE emission order is
  proj(i) -> V-transpose(i) -> outproj(i-1) -> attention(i)
so the tensor engine never crosses a phase barrier (keeps HAM warm).
RoPE is software-pipelined (swap-DMA issued at evac, cos/sin muls one pass
later) so the DVE never blocks on DMA latency and PSUM banks recycle
promptly; attn(3) holds back 3 outproj(2) fillers to cover the final
flush->reciprocal->mul chain; the last chunk's stores are per-co DMAs.
Projection runs one output at a time (k, q0..q3, v: 16-chunk accumulation
chains in a single PSUM bank each) so projections need only the 2 shared
"generic" PSUM banks; attention uses 2-bank score pairs (one exp per block
pair), accumulating den (ones-matmul) and O over s-blocks; softmax denom
reciprocal via the fast approx DVE op. All DRAM inputs are host-pre-tiled
to match SBUF layouts so every DMA is contiguous; outputs are fp16
partials summed on the host.
"""

import sys

sys.path.insert(0, "/opt/trn_rl_repo")

import numpy as np

import concourse.bass as bass
import concourse.mybir as mybir
import concourse.tile as tile
from concourse import bacc
from concourse.bass_utils import run_bass_kernel_spmd
from concourse.masks import make_identity

F32 = mybir.dt.float32
F16 = mybir.dt.float16
FP8 = mybir.dt.float8e4
AF = mybir.ActivationFunctionType
DR = mybir.MatmulPerfMode.DoubleRow
DP = mybir.MatmulPerfMode.DoublePixel

B, T, C = 2, 2048, 2048
N_HEAD, N_KV_HEAD = 16, 4
HD = 128                 # head dim
QH = 4                   # q heads per core
TQ = 512                 # t-chunk
NT = T // TQ             # 4 t-chunks
CK = C // 128            # 16 contraction chunks of 128
SCALE = 1.0 / float(np.sqrt(HD))
# exp is computed as exp(score*SCALE - ln8) = exp(score*SCALE)/8; the /8
# cancels in O/den but keeps the fp8e4 den copy far from TRN-fp8's +-240
# max (values >=248 cast to inf -> NaN through the reciprocal).
EXP_BIAS = -float(np.log(8.0))
MASK_NEG = -1e30

_CACHE = {}


def _build_nc():
    nc = bacc.Bacc("TRN2", target_bir_lowering=False, debug=False, num_devices=8)

    # All inputs pre-tiled on host so DRAM layout == SBUF layout.
    xH = nc.dram_tensor("xH", [NT, 128, CK, TQ], F16, kind="ExternalInput").ap()
    wqH = nc.dram_tensor("wqH", [128, CK, QH * HD], F16, kind="ExternalInput").ap()
    wkH = nc.dram_tensor("wkH", [128, CK, HD], F16, kind="ExternalInput").ap()
    wvH = nc.dram_tensor("wvH", [128, CK, HD], F16, kind="ExternalInput").ap()
    woH = nc.dram_tensor("woH", [128, CK, QH * HD], F16, kind="ExternalInput").ap()
    cosH = nc.dram_tensor("cosH", [HD, T], F16, kind="ExternalInput").ap()
    sinH = nc.dram_tensor("sinH", [HD, T], F16, kind="ExternalInput").ap()
    # fp8 copies for the chunk>=1 q/k projections (x pre-scaled by 4, w by
    # 256 so weights sit in fp8e4's normal range; evac divides by 1024)
    x8H = nc.dram_tensor("x8H", [NT, 128, CK, TQ], FP8, kind="ExternalInput").ap()
    wq8H = nc.dram_tensor("wq8H", [128, CK, QH * HD], FP8,
                          kind="ExternalInput").ap()
    wk8H = nc.dram_tensor("wk8H", [128, CK, HD], FP8, kind="ExternalInput").ap()
    outX = nc.dram_tensor("outX", [NT, 128, CK, TQ], F16, kind="ExternalOutput").ap()

    with tile.TileContext(nc) as tc:
        _emit(nc, tc, xH, wqH, wkH, wvH, woH, cosH, sinH,
              x8H, wq8H, wk8H, outX)

    nc.compile()
    return nc


PROJ_UNSCALE = 1.0 / 1024.0


def _emit(nc, tc, xH, wqH, wkH, wvH, woH, cosH, sinH, x8H, wq8H, wk8H, outX):
    import contextlib

    ctx = contextlib.ExitStack()
    with ctx:
        singles = ctx.enter_context(tc.tile_pool(name="singles", bufs=1))

        # ---- resident tiles ----
        wq_sb = singles.tile([128, CK, QH * HD], F16)
        wk_sb = singles.tile([128, CK, HD], F16)
        wv_sb = singles.tile([128, CK, HD], F16)
        wo_sb = singles.tile([128, CK, QH * HD], F16)
        wq8_sb = singles.tile([128, CK, QH * HD], FP8)
        wk8_sb = singles.tile([128, CK, HD], FP8)
        cos_sb = singles.tile([HD, T], F16)
        sin_sb = singles.tile([HD, T], F16)

        qT_sb = singles.tile([128, QH, T], F16)    # per head [dq, t], RoPE'd
        kT_sb = singles.tile([128, T], F16)        # [dk, t], RoPE'd
        v_sb = singles.tile([128, CK, HD], F16)    # [s in blk, (blk, dv)]
        oT_sb = singles.tile([128, QH, T], F16)    # per head [dv, t] normalized

        ident = singles.tile([128, 128], F32)
        cmask = singles.tile([128, 128], F32)
        ones_sq = singles.tile([128, 128], F16)
        ones2_8 = singles.tile([128, 2, 128], FP8)
        ebias = singles.tile([128, 1], F32)

        # ---- pools ----
        xpool = ctx.enter_context(tc.tile_pool(name="xpool", bufs=2))
        x8pool = ctx.enter_context(tc.tile_pool(name="x8pool", bufs=2))
        ppool = ctx.enter_context(tc.tile_pool(name="ppool", bufs=3))
        vtsb = ctx.enter_context(tc.tile_pool(name="vtsb", bufs=2))
        rope = ctx.enter_context(tc.tile_pool(name="rope", bufs=3))
        invp = ctx.enter_context(tc.tile_pool(name="invp", bufs=2))
        outsb = ctx.enter_context(tc.tile_pool(name="outsb", bufs=2))
        ps_s = ctx.enter_context(tc.tile_pool(name="ps_s", bufs=2, space="PSUM"))
        ps_d = ctx.enter_context(tc.tile_pool(name="ps_d", bufs=1, space="PSUM"))
        ps_o = ctx.enter_context(tc.tile_pool(name="ps_o", bufs=1, space="PSUM"))
        ps_g = ctx.enter_context(tc.tile_pool(name="ps_g", bufs=2, space="PSUM"))

        # ---- startup DMAs, criticality-ordered ----
        x_t = [None] * NT
        x8_t = [None] * NT

        def load_x(i):
            x_t[i] = xpool.tile([128, CK, TQ], F16, tag="x", name=f"x{i}")
            for a in range(2):
                nc.sync.dma_start(out=x_t[i][:, 8 * a:8 * a + 8, :],
                                  in_=xH[i, :, 8 * a:8 * a + 8, :])

        def load_x8(i):
            x8_t[i] = x8pool.tile([128, CK, TQ], FP8, tag="x8", name=f"x8_{i}")
            for a in range(2):
                nc.sync.dma_start(out=x8_t[i][:, 8 * a:8 * a + 8, :],
                                  in_=x8H[i, :, 8 * a:8 * a + 8, :])

        # Startup DMAs: each dma_start costs ~600ns of ISSUE time on its
        # engine (transfers then run async), and nothing issues before the
        # engine programs load (~7us).  So: few, coarse pieces, split across
        # BOTH hw-DGE engines (sync + scalar) so the two issue queues run in
        # parallel.  sync gets the critical-path x0/wk; scalar gets wq and
        # the rest.
        nc.sync.dma_start(out=wk_sb, in_=wkH)
        x_t[0] = xpool.tile([128, CK, TQ], F16, tag="x", name="x0")
        for a in range(4):
            nc.sync.dma_start(out=x_t[0][:, 4 * a:4 * a + 4, :],
                              in_=xH[0, :, 4 * a:4 * a + 4, :])
        for a in range(2):
            nc.scalar.dma_start(out=wq_sb[:, 8 * a:8 * a + 8, :],
                                in_=wqH[:, 8 * a:8 * a + 8, :])
        nc.scalar.dma_start(out=cos_sb, in_=cosH)
        nc.scalar.dma_start(out=sin_sb, in_=sinH)
        nc.scalar.dma_start(out=wv_sb, in_=wvH)
        nc.scalar.dma_start(out=wq8_sb, in_=wq8H)
        nc.scalar.dma_start(out=wk8_sb, in_=wk8H)
        load_x8(1)  # needed by attn(0)'s chunk-1 k/q0 fillers
        load_x(1)
        nc.sync.dma_start(out=wo_sb, in_=woH)

        make_identity(nc, ident)
        nc.gpsimd.memset(cmask, 0.0)
        nc.gpsimd.affine_select(
            out=cmask, in_=cmask, compare_op=mybir.AluOpType.is_ge,
            fill=MASK_NEG, base=0, pattern=[[1, 128]], channel_multiplier=-1,
        )
        nc.vector.memset(ones_sq, 1.0)
        nc.vector.tensor_copy(out=ones2_8[:, 0, :], in_=ones_sq)
        nc.vector.tensor_copy(out=ones2_8[:, 1, :], in_=ones_sq)
        nc.vector.memset(ebias, EXP_BIAS)

        # HAM pre-warm: dummy matmuls covering the initial DMA ramp, so real
        # matmuls run at 2.4 GHz from the start with no re-throttle.
        for w in range(2):
            warm = ps_g.tile([128, 128], F32, tag="g", name=f"warm{w}")
            for _ in range(20):
                nc.tensor.matmul(warm, ones_sq, ones_sq, start=True, stop=True)

        # RoPE is software-pipelined: start_rope issues the half-swap DMAs,
        # finish_rope (called one proj pass later, when the DMA has long
        # landed) does the cos/sin muls.  This keeps the DVE from blocking on
        # DMA latency, so the next pass's PSUM-evac copy isn't queued behind a
        # stalled mul and the PE never waits on a ps_g bank.
        rope_pend = []

        def start_rope(tgt, ti, t8=None):
            sw = rope.tile([128, TQ], F16, tag="swap")
            nc.sync.dma_start(out=sw[0:64, :], in_=tgt[64:128, :])
            nc.sync.dma_start(out=sw[64:128, :], in_=tgt[0:64, :])
            rope_pend.append((tgt, sw, ti, t8))

        def finish_rope(force=False):
            # only pop entries whose swap-DMA has had a full pass to land,
            # unless force (phase end)
            if not rope_pend or (not force and len(rope_pend) < 2):
                return
            tgt, sw, ti, t8 = rope_pend.pop(0)
            tmp = rope.tile([128, TQ], F16, tag="tmp")
            nc.vector.tensor_mul(tmp, tgt, cos_sb[:, ti:ti + TQ])
            nc.vector.tensor_mul(sw, sw, sin_sb[:, ti:ti + TQ])
            nc.vector.tensor_add(tgt, tmp, sw)
            if t8 is not None:
                nc.vector.tensor_scalar_mul(t8, tgt, 16.0)

        def proj_pass(i, w_sb, col0, ncol, kind, h=None):
            """One projection output over all 16 c-chunks into 1 PSUM bank.
            q/k passes for chunks >=1 run in fp8 DoubleRow (2 contraction
            tiles per pass, 2x PE throughput); every row they produce is
            consumed with >=512 softmax terms so the quantization noise
            averages out.  fp8 x is pre-scaled by 4 and w by 256, undone at
            evac."""
            ti = TQ * i
            acc = ps_g.tile([128, TQ], F32, tag="g")
            fp8 = kind in ("k", "q") and i >= 1
            if fp8:
                w8 = wk8_sb if kind == "k" else wq8_sb
                for kk in range(0, CK, 2):
                    nc.tensor.matmul(acc, w8[:, kk:kk + 2, col0:col0 + ncol],
                                     x8_t[i][:, kk:kk + 2, :],
                                     start=(kk == 0), stop=(kk == CK - 2),
                                     perf_mode=DR)
            else:
                for kk in range(CK):
                    nc.tensor.matmul(acc, w_sb[:, kk, col0:col0 + ncol],
                                     x_t[i][:, kk, :],
                                     start=(kk == 0), stop=(kk == CK - 1))

            def evac(tgt):
                if fp8:
                    # scalar engine is idle in standalone proj phases, and
                    # fp8 passes are short enough that the DVE (evac + rope)
                    # would otherwise become the pipeline limiter
                    nc.scalar.activation(tgt, acc, AF.Copy,
                                         scale=PROJ_UNSCALE)
                else:
                    nc.vector.tensor_copy(out=tgt, in_=acc)

            if kind == "k":
                evac(kT_sb[:, ti:ti + TQ])
                start_rope(kT_sb[:, ti:ti + TQ], ti)
                finish_rope()
                return None
            if kind == "q":
                evac(qT_sb[:, h, ti:ti + TQ])
                start_rope(qT_sb[:, h, ti:ti + TQ], ti)
                finish_rope()
                return None
            vt = vtsb.tile([128, TQ], F32, tag="vt")
            nc.vector.tensor_copy(out=vt, in_=acc)
            finish_rope()
            return vt

        def drain_rope():
            while rope_pend:
                finish_rope(force=True)

        def vts(i, vt):
            """V^T -> natural [s, dv] blocks via PE transpose."""
            for jj in range(TQ // 128):
                vt_ps = ps_g.tile([128, 128], F32, tag="g")
                nc.tensor.transpose(vt_ps, vt[:, 128 * jj:128 * (jj + 1)], ident)
                nc.vector.tensor_copy(out=v_sb[:, 4 * i + jj, :], in_=vt_ps)

        def outproj_units(i, fine_dma=False):
            """16 co-block emitters for output projection of t-chunk i;
            used as PE gap-filler inside the next chunk's attention.
            fine_dma: store per-co (smaller DMAs) so the final store after
            the last matmul is short — use for the last chunk's tail."""
            ti = TQ * i
            grp = [None]

            def unit(co):
                def emit():
                    if co % 4 == 0:
                        grp[0] = outsb.tile([128, 4, TQ], F16, tag="ot",
                                            bufs=3, name=f"osb{i}_{co // 4}")
                    osb = grp[0]
                    ot = ps_g.tile([128, TQ], F32, tag="g")
                    for h in range(QH):
                        nc.tensor.matmul(ot, wo_sb[:, co, HD * h:HD * (h + 1)],
                                         oT_sb[:, h, ti:ti + TQ],
                                         start=(h == 0), stop=(h == QH - 1))
                    nc.vector.tensor_copy(out=osb[:, co % 4, :], in_=ot)
                    if fine_dma:
                        nc.sync.dma_start(
                            out=outX[i, :, co:co + 1, :],
                            in_=osb[:, co % 4:co % 4 + 1, :])
                    elif co % 4 == 3:  # store per 4-co group
                        a = co // 4
                        nc.sync.dma_start(out=outX[i, :, 4 * a:4 * a + 4, :],
                                          in_=osb)
                return emit
            return [unit(co) for co in range(CK)]

        def proj_units(i, passes):
            """q/k projection passes (fp8 DoubleRow, for chunks >=1) split
            into 2-matmul chain steps, usable as attention gap-filler.
            passes: list of (kind, h)."""
            assert i >= 1
            units = []
            pend_fin = False
            for kind, h in passes:
                w8, col0 = (wk8_sb, 0) if kind == "k" else (wq8_sb, HD * h)
                acc = ps_g.tile([128, TQ], F32, tag="g",
                                name=f"acc{i}_{kind}{h}")

                def step(acc, w8, col0, g0):
                    def emit():
                        for kk in range(g0, g0 + 4, 2):
                            nc.tensor.matmul(
                                acc, w8[:, kk:kk + 2, col0:col0 + HD],
                                x8_t[i][:, kk:kk + 2, :],
                                start=(kk == 0), stop=(kk == CK - 2),
                                perf_mode=DR)
                    return emit

                def evac(acc, kind, h, ti):
                    def emit():
                        if kind == "k":
                            tgt = kT_sb[:, ti:ti + TQ]
                        else:
                            tgt = qT_sb[:, h, ti:ti + TQ]
                        nc.vector.tensor_scalar_mul(tgt, acc, PROJ_UNSCALE)
                        start_rope(tgt, ti)
                    return emit

                for g0 in range(0, CK, 4):
                    units.append(step(acc, w8, col0, g0))
                    if pend_fin and g0 == 0:
                        # finish the PREVIOUS pass's rope one step into this
                        # pass, giving its swap-DMA time to land
                        units.append(lambda: finish_rope(force=True))
                        pend_fin = False
                units.append(evac(acc, kind, h, TQ * i))
                pend_fin = True
            if pend_fin:
                units.append(lambda: finish_rope(force=True))
            return units

        def attn_chunk(i, fillers, holdback=0):
            """Attention for t-chunk i: flat pair-stream over (head, pair)
            with one-item lookahead so PE rarely waits on exp; `fillers`
            (outproj co-blocks or next-chunk proj steps) are interleaved
            evenly to cover exp latency with useful matmuls.  The last
            `holdback` fillers are reserved until after the final flush so
            the PE has work during the last flush->reciprocal->mul chain
            (only matters when the NEXT phase depends on this chunk's oT)."""
            ti = TQ * i
            nj = 4 * (i + 1)
            npair = nj // 2
            n_items = QH * npair
            fill_idx = 0
            n_distr = len(fillers) - holdback

            def blk(j):
                t0 = max(ti, 128 * j)
                return t0, TQ * (i + 1) - t0, t0 - ti  # t0, N, c0

            acc_t = {}  # h -> (den, o_ps)

            def flush(h, p, pp, dr_info, blocks):
                if p == 0:
                    den = ps_d.tile([128, TQ], F32, tag="d",
                                    name=f"den{i}_{h}")
                    o_ps = ps_o.tile([128, TQ], F32, tag="o",
                                     name=f"o{i}_{h}")
                    acc_t[h] = (den, o_ps)
                den, o_ps = acc_t[h]
                first, last = (p == 0), (p == npair - 1)
                if dr_info is not None:
                    # fp8 DoubleRow den: one matmul sums both 128-blocks at
                    # once over their common columns (den err averages out
                    # over the >=512 softmax terms chunk>=1 rows have);
                    # diagonal pairs add an f16 matmul for block j0's
                    # leading columns
                    pp8, n1, dstc, c0l = dr_info
                    if n1 < TQ:
                        nc.tensor.matmul(den[:, c0l:c0l + 128], ones_sq,
                                         pp[:, 0:128],
                                         start=first, stop=False)
                        nc.tensor.matmul(den[:, dstc:dstc + n1], ones2_8,
                                         pp8[:, :, 0:n1],
                                         start=False, stop=last,
                                         perf_mode=DR)
                    else:
                        nc.tensor.matmul(den[:, 0:TQ], ones2_8, pp8,
                                         start=first, stop=False,
                                         perf_mode=DR)
                else:
                    for bi, (j, loc, N, c0) in enumerate(blocks):
                        st = first and bi == 0
                        sp = last and bi == len(blocks) - 1
                        nc.tensor.matmul(den[:, c0:c0 + N], ones_sq,
                                         pp[:, loc:loc + N], start=st, stop=sp)
                for bi, (j, loc, N, c0) in enumerate(blocks):
                    st = first and bi == 0
                    sp = last and bi == len(blocks) - 1
                    nc.tensor.matmul(o_ps[:, c0:c0 + N], v_sb[:, j, :],
                                     pp[:, loc:loc + N], start=st, stop=sp)
                if last:
                    inv = invp.tile([128, TQ], F32, tag="inv")
                    nc.vector.reciprocal_approx_fast(out=inv, in_=den)
                    nc.vector.tensor_mul(oT_sb[:, h, ti:ti + TQ], o_ps, inv)

            pend = None
            n = 0
            for h in range(QH):
                for p in range(npair):
                    j0, j1 = 2 * p, 2 * p + 1
                    t0a, N0, c0a = blk(j0)
                    t0b, N1, c0b = blk(j1)
                    sp_t = ps_s.tile([128, 2 * TQ], F32, tag="s")
                    nc.tensor.matmul(sp_t[:, 0:N0],
                                     kT_sb[:, 128 * j0:128 * (j0 + 1)],
                                     qT_sb[:, h, t0a:t0a + N0],
                                     start=True, stop=True)
                    nc.tensor.matmul(sp_t[:, TQ:TQ + N1],
                                     kT_sb[:, 128 * j1:128 * (j1 + 1)],
                                     qT_sb[:, h, t0b:t0b + N1],
                                     start=True, stop=True)
                    if j0 >= 4 * i:  # diagonal blocks: causal mask
                        nc.vector.tensor_add(sp_t[:, 0:128],
                                             sp_t[:, 0:128], cmask)
                    if j1 >= 4 * i:
                        nc.vector.tensor_add(sp_t[:, TQ:TQ + 128],
                                             sp_t[:, TQ:TQ + 128], cmask)
                    pp = ppool.tile([128, 2 * TQ], F16, tag="p")
                    ncols = TQ + N1
                    nc.scalar.activation(pp[:, :ncols], sp_t[:, :ncols],
                                         AF.Exp, scale=SCALE, bias=ebias)
                    dr_info = None
                    if j1 < 4 * i:  # off-diagonal pair: fp8 copy for den
                        pp8 = ppool.tile([128, 2, TQ], FP8, tag="p8")
                        nc.vector.tensor_copy(out=pp8[:, 0, :],
                                              in_=pp[:, 0:TQ])
                        nc.vector.tensor_copy(out=pp8[:, 1, :],
                                              in_=pp[:, TQ:2 * TQ])
                        dr_info = (pp8, TQ, 0, 0)
                    if pend is not None:
                        flush(*pend)
                    pend = (h, p, pp, dr_info,
                            [(j0, 0, N0, c0a), (j1, TQ, N1, c0b)])
                    n += 1
                    while fill_idx * n_items < n * n_distr:
                        fillers[fill_idx]()
                        fill_idx += 1
            # run most held-back fillers BEFORE the final flush (they cover
            # the last exp's latency), keep one for the reciprocal+mul tail
            while fill_idx < len(fillers) - 1 and holdback > 0:
                fillers[fill_idx]()
                fill_idx += 1
            flush(*pend)
            while fill_idx < len(fillers):
                fillers[fill_idx]()
                fill_idx += 1

        def full_proj(i):
            """All projections for chunk i, V-transposes mid-way so they
            don't queue behind all the RoPE work on the DVE."""
            proj_pass(i, wk_sb, 0, HD, "k")
            proj_pass(i, wq_sb, 0, HD, "q", h=0)
            proj_pass(i, wq_sb, HD, HD, "q", h=1)
            vt = proj_pass(i, wv_sb, 0, HD, "v")
            vts(i, vt)
            proj_pass(i, wq_sb, 2 * HD, HD, "q", h=2)
            proj_pass(i, wq_sb, 3 * HD, HD, "q", h=3)
            drain_rope()

        # ======== fused pipeline ========
        # chunk 0 projections, then attn(0) filled with proj(1) k/q0 steps,
        # then the rest of proj(1), then attn(i) filled with outproj(i-1).
        full_proj(0)
        attn_chunk(0, proj_units(1, [("k", None), ("q", 0)]))
        proj_pass(1, wq_sb, HD, HD, "q", h=1)
        vt = proj_pass(1, wv_sb, 0, HD, "v")
        vts(1, vt)
        proj_pass(1, wq_sb, 2 * HD, HD, "q", h=2)
        proj_pass(1, wq_sb, 3 * HD, HD, "q", h=3)
        drain_rope()
        load_x8(2)
        load_x(2)
        attn_chunk(1, outproj_units(0))
        full_proj(2)
        load_x8(3)
        load_x(3)
        attn_chunk(2, outproj_units(1))
        full_proj(3)
        attn_chunk(3, outproj_units(2), holdback=5)
        for u in outproj_units(3, fine_dma=True):
            u()
        # tail warmers: keep the PE active while the last evac copies/DMAs
        # drain, so HAM doesn't down-clock and stretch the drain sequence
        tailw = ps_g.tile([128, 128], F32, tag="g", name="tailw")
        for _ in range(48):
            nc.tensor.matmul(tailw, ones_sq, ones_sq, start=True, stop=True)


_PERM = np.concatenate([np.arange(0, HD, 2), np.arange(1, HD, 2)])

PROFILE = False
LAST_EXEC_NS = None
LAST_RESULTS = None


def kernel(x, freqs_cos, freqs_sin, wq, wk, wv, wo):
    global LAST_EXEC_NS, LAST_RESULTS
    if "nc" not in _CACHE:
        _CACHE["nc"] = _build_nc()
    nc = _CACHE["nc"]

    x = np.asarray(x, dtype=np.float32)
    fc = np.asarray(freqs_cos, dtype=np.float32)
    fs = np.asarray(freqs_sin, dtype=np.float32)
    wq = np.asarray(wq, dtype=np.float32)
    wk = np.asarray(wk, dtype=np.float32)
    wv = np.asarray(wv, dtype=np.float32)
    wo = np.asarray(wo, dtype=np.float32)

    cosT = fc.T                                   # [64, T]
    sinT = fs.T
    cosH = np.ascontiguousarray(
        np.concatenate([cosT, cosT], axis=0).astype(np.float16))   # [128, T]
    sinH = np.ascontiguousarray(
        np.concatenate([-sinT, sinT], axis=0).astype(np.float16))

    import ml_dtypes
    E4 = ml_dtypes.float8_e4m3

    in_maps = []
    for core in range(8):
        b, g = core // 4, core % 4
        xT32 = x[b].T                                         # [C, T] f32
        xT = xT32.astype(np.float16)
        # [C, T] -> [NT, 128(p), CK(k), TQ]: xH[i, p, k, t] = xT[128k+p, 512i+t]
        xH = np.ascontiguousarray(
            xT.reshape(CK, 128, NT, TQ).transpose(2, 1, 0, 3))
        # fp8 copy, pre-scaled by 4 (see kernel comment)
        x8H = np.ascontiguousarray(
            (xT32 * 4.0).reshape(CK, 128, NT, TQ).transpose(2, 1, 0, 3)
        ).astype(E4)
        wq_g = wq[512 * g:512 * (g + 1)].reshape(QH, HD, C)[:, _PERM, :]
        wqT32 = wq_g.reshape(QH * HD, C).T                    # [C, 512] f32
        wqT = wqT32.astype(np.float16)
        wqH = np.ascontiguousarray(
            wqT.reshape(CK, 128, QH * HD).transpose(1, 0, 2))  # [128, CK, 512]
        wq8H = np.ascontiguousarray(
            (wqT32 * 256.0).reshape(CK, 128, QH * HD).transpose(1, 0, 2)
        ).astype(E4)
        wkT32 = wk[HD * g:HD * (g + 1)][_PERM].T              # [C, 128] f32
        wkT = wkT32.astype(np.float16)
        wkH = np.ascontiguousarray(wkT.reshape(CK, 128, HD).transpose(1, 0, 2))
        wk8H = np.ascontiguousarray(
            (wkT32 * 256.0).reshape(CK, 128, HD).transpose(1, 0, 2)
        ).astype(E4)
        wvT = wv[HD * g:HD * (g + 1)].T.astype(np.float16)
        wvH = np.ascontiguousarray(wvT.reshape(CK, 128, HD).transpose(1, 0, 2))
        wo_g = wo[:, 512 * g:512 * (g + 1)]                   # [C, 512]
        # woH[p, co, 128h+d] = wo[128co+d, 512g+128h+p]
        woH = np.ascontiguousarray(
            wo_g.reshape(CK, 128, QH, 128).transpose(3, 0, 2, 1)
        ).astype(np.float16).reshape(128, CK, QH * 128)
        in_maps.append({
            "xH": xH, "wqH": wqH, "wkH": wkH, "wvH": wvH, "woH": woH,
            "x8H": x8H, "wq8H": wq8H, "wk8H": wk8H,
            "cosH": cosH, "sinH": sinH,
        })

    res = run_bass_kernel_spmd(nc, in_maps, list(range(8)), trace=PROFILE)
    LAST_EXEC_NS = res.exec_time_ns
    LAST_RESULTS = res

    out = np.empty((B, T, C), dtype=np.float32)
    for b in range(B):
        acc = res.results[4 * b]["outX"].astype(np.float32)
        for g in range(1, 4):
            acc = acc + res.results[4 * b + g]["outX"]
        # outX[i, d?, co, t]: out[b][512i+t, 128co+d] = outX[i, d, co, t]
        out[b] = acc.transpose(0, 3, 2, 1).reshape(T, C)
    return out

